# revision 1
# baseline (speedup 1.0000x reference)
"""BitFeedForward (Hadamard + int8 act-quant + ternary weights) on 8 TRN2 cores.

Data-parallel over tokens (8192 -> 1024/core). v2:
  - weight ternarization streamed on ACT, overlapped with the x-path
  - per-token absmax via gpsimd partition_all_reduce(absmax) broadcast scales
  - in-place butterflies (a += b; b = a' - 2b) -> one live FWHT tile per block
  - FWHT1 in token-halves: DVE chain (half 0) vs GPSIMD chain (half 1)
  - FWHT2 whole-block; stages 2-5 split DVE/GPSIMD by chunk-group subranges
  - t1 resident in SBUF; t2 streamed from DRAM in 1/8 slabs, restreamed per j
"""
import math
import numpy as np
from contextlib import ExitStack

import concourse.bass as bass
from concourse import bacc
import concourse.tile as tile
import concourse.mybir as mybir
import concourse.bass_isa as bass_isa
from concourse.bass_utils import run_bass_kernel_spmd
from concourse.masks import make_identity

F32 = mybir.dt.float32
BF16 = mybir.dt.bfloat16
FP8 = mybir.dt.float8e4

NCORES = 8
B, S, H, I = 4, 2048, 2048, 4096
TOKENS = B * S
T = TOKENS // NCORES
TB = 256
NB = T // TB
TH = 128
NC1 = H // 128
NC2 = I // 128
C_MAGIC = 12582912.0
ISQ1 = 1.0 / math.sqrt(H)
WCOUNT = float(H * I)

ADD = mybir.AluOpType.add
SUB = mybir.AluOpType.subtract
MULT = mybir.AluOpType.mult
MAX = mybir.AluOpType.max
AF = mybir.ActivationFunctionType
RMAX = bass_isa.ReduceOp.absmax


def _bfly_ip(eng, t, nchunk, sigma, span, lo=0.0, hi=1.0, is_gp=False):
    """In-place butterfly over the chunk axis: a' = a+b ; b' = a' - 2b.

    t: tile [128, nchunk*span]; chunk c at cols [c*span,(c+1)*span).
    lo/hi select a fraction of the s-subrange for engine splitting.
    """
    v = t[:].rearrange("p (g two s) -> p g two s", two=2, s=sigma * span)
    n = sigma * span
    s0, s1 = int(n * lo), int(n * hi)
    if s1 <= s0:
        return
    a = v[:, :, 0, s0:s1]
    bv = v[:, :, 1, s0:s1]
    eng.tensor_tensor(a, a, bv, ADD)
    if is_gp:
        # Pool engine lacks scalar_tensor_tensor: b = 2b, then b = a' - b
        eng.tensor_tensor(bv, bv, bv, ADD)
        eng.tensor_tensor(bv, a, bv, SUB)
    else:
        eng.scalar_tensor_tensor(bv, bv, -2.0, a, MULT, ADD)


def build():
    nc = bacc.Bacc()
    x_in = nc.declare_dram_parameter("x", [T, H], F32, isOutput=False)
    wuT_in = nc.declare_dram_parameter("wuT", [H, I], F32, isOutput=False)
    wdT_in = nc.declare_dram_parameter("wdT", [I, H], F32, isOutput=False)
    wus_in = nc.declare_dram_parameter("wu_s", [H // NCORES, I], F32, isOutput=False)
    wds_in = nc.declare_dram_parameter("wd_s", [I // NCORES, H], F32, isOutput=False)
    h128_in = nc.declare_dram_parameter("h128", [128, 128], F32, isOutput=False)
    out_d = nc.declare_dram_parameter("out", [T, H], F32, isOutput=True)

    cc_in = nc.dram_tensor("cc_in", [1, 2], F32)
    cc_out = nc.dram_tensor("cc_out", [1, 2], F32, addr_space="Shared")

    with tile.TileContext(nc) as tc, ExitStack() as ctx:
        const = ctx.enter_context(tc.tile_pool(name="const", bufs=1))
        t1p = ctx.enter_context(tc.tile_pool(name="t1", bufs=1))
        wq = ctx.enter_context(tc.tile_pool(name="wq", bufs=2))
        t2tp = ctx.enter_context(tc.tile_pool(name="t2t", bufs=1))
        slab = ctx.enter_context(tc.tile_pool(name="slab", bufs=2))
        xinp = ctx.enter_context(tc.tile_pool(name="xinp", bufs=2))
        fw1 = ctx.enter_context(tc.tile_pool(name="fw1", bufs=2))
        q1p = ctx.enter_context(tc.tile_pool(name="q1", bufs=1))
        fw2 = ctx.enter_context(tc.tile_pool(name="fw2", bufs=2))
        q2p = ctx.enter_context(tc.tile_pool(name="q2", bufs=1))
        otp = ctx.enter_context(tc.tile_pool(name="otp", bufs=2))
        scl = ctx.enter_context(tc.tile_pool(name="scl", bufs=1))
        ccp = ctx.enter_context(tc.tile_pool(name="ccp", bufs=2))
        tiny = ctx.enter_context(tc.tile_pool(name="tiny", bufs=2))
        dram = ctx.enter_context(tc.tile_pool(name="dram", bufs=1, space="DRAM"))
        ps_x = ctx.enter_context(tc.tile_pool(name="ps_x", bufs=1, space="PSUM"))
        ps_h2 = ctx.enter_context(tc.tile_pool(name="ps_h2", bufs=1, space="PSUM"))
        ps_g1 = ctx.enter_context(tc.tile_pool(name="ps_g1", bufs=2, space="PSUM"))
        ps_g2 = ctx.enter_context(tc.tile_pool(name="ps_g2", bufs=4, space="PSUM"))

        ident = const.tile([128, 128], F32)
        make_identity(nc, ident[:])
        h128 = const.tile([128, 128], F32)
        nc.sync.dma_start(h128[:], h128_in[:])
        negh = const.tile([128, 128], F32)
        nc.vector.tensor_scalar(negh[:], h128[:], -1.0, None, MULT)
        ones_col = const.tile([128, 1], F32)
        nc.vector.memset(ones_col[:], 1.0)
        biasC = const.tile([128, 1], F32)
        nc.vector.memset(biasC[:], C_MAGIC)
        biasNC = const.tile([128, 1], F32)
        nc.vector.memset(biasNC[:], -C_MAGIC)

        t2_dram = []
        for _tq in range(4):
            t2q = dram.tile([I // 4, H], FP8, tag=f"t2q{_tq}")
            t2_dram.append(t2q)

        # ---- weight scale partials + AllReduce ----
        def slice_partials(src, nchunk, width, tagp):
            accs = []
            for i in range(nchunk):
                for q in range(width // 1024):
                    ch = wq.tile([128, 1024], F32, tag="wq")
                    nc.sync.dma_start(
                        ch[:], src[i * 128:(i + 1) * 128, q * 1024:(q + 1) * 1024])
                    acc = tiny.tile([128, 1], F32, tag="wacc")
                    nc.vector.tensor_reduce(
                        acc[:], ch[:], mybir.AxisListType.X, ADD,
                        apply_absolute_value=True)
                    accs.append(acc)
            total = tiny.tile([128, 1], F32, tag=f"{tagp}tot")
            nc.vector.tensor_add(total[:], accs[0][:], accs[1][:])
            for a in accs[2:]:
                nc.vector.tensor_add(total[:], total[:], a[:])
            return total

        pu = slice_partials(wus_in, 2, I, "au")
        pd = slice_partials(wds_in, 4, H, "ad")
        psum2 = ps_g2.tile([1, 2], F32, tag="a2")
        nc.tensor.matmul(psum2[:, 0:1], pu[:], ones_col[:], start=True, stop=True)
        nc.tensor.matmul(psum2[:, 1:2], pd[:], ones_col[:], start=True, stop=True)
        part = tiny.tile([1, 2], F32)
        nc.vector.tensor_copy(part[:], psum2[:])

        res2 = tiny.tile([1, 2], F32)
        dsem = nc.alloc_semaphore("cc_dma")
        csem = nc.alloc_semaphore("cc_done")
        with tc.tile_critical():
            nc.gpsimd.dma_start(cc_in[:, :], part[:]).then_inc(dsem, 16)
            nc.gpsimd.wait_ge(dsem, 16)
            nc.gpsimd.collective_compute(
                "AllReduce", ADD,
                replica_groups=[list(range(NCORES))],
                ins=[cc_in[:, :]], outs=[cc_out[:, :]],
            ).then_inc(csem)
            nc.gpsimd.wait_ge(csem, 1)
            nc.gpsimd.dma_start(res2[:], cc_out[:, :]).then_inc(dsem, 32)
            nc.gpsimd.wait_ge(dsem, 48)

        srow = tiny.tile([1, 2], F32)
        nc.vector.tensor_scalar(srow[:], res2[:], 1.0 / WCOUNT, 1e-5, MULT, MAX)
        sW = tiny.tile([128, 2], F32)
        nc.gpsimd.partition_broadcast(sW[:], srow[:], 128)
        rW = tiny.tile([128, 2], F32)
        nc.vector.reciprocal(rW[:], sW[:])

        t1 = t1p.tile([128, NC1 * I], FP8)  # resident ternary w_up^T

        def ternarize_t1(kc):
            for q in range(4):
                ch = wq.tile([128, 1024], F32, tag="wq")
                nc.sync.dma_start(
                    ch[:], wuT_in[kc * 128:(kc + 1) * 128, q * 1024:(q + 1) * 1024])
                for h_ in range(2):
                    tmp = ps_h2.tile([128, 512], F32, tag="h2")
                    nc.scalar.activation(tmp[:], ch[:, h_ * 512:(h_ + 1) * 512],
                                         AF.Identity, bias=biasC[:], scale=rW[:, 0:1])
                    nc.scalar.activation(
                        t1[:, kc * I + q * 1024 + h_ * 512:
                           kc * I + q * 1024 + (h_ + 1) * 512],
                        tmp[:], AF.Sign, bias=biasNC[:], scale=1.0)

        def ternarize_t2(mc):
            t2t = t2tp.tile([128, H], FP8, tag="t2t")
            for q in range(2):
                ch = wq.tile([128, 1024], F32, tag="wq")
                nc.sync.dma_start(
                    ch[:], wdT_in[mc * 128:(mc + 1) * 128, q * 1024:(q + 1) * 1024])
                for h_ in range(2):
                    tmp = ps_h2.tile([128, 512], F32, tag="h2")
                    nc.scalar.activation(tmp[:], ch[:, h_ * 512:(h_ + 1) * 512],
                                         AF.Identity, bias=biasC[:], scale=rW[:, 1:2])
                    nc.scalar.activation(
                        t2t[:, q * 1024 + h_ * 512:q * 1024 + (h_ + 1) * 512],
                        tmp[:], AF.Sign, bias=biasNC[:], scale=1.0)
            nc.sync.dma_start(t2_dram[mc // 8][(mc % 8) * 128:(mc % 8 + 1) * 128, :],
                              t2t[:])

        def x_half(b, j, q1, ccw):
            """FWHT1 + quant for token-half j of block b. DVE chain for j=0,
            GPSIMD chain for j=1 (stage 1 via ACT evac since GPSIMD can't PSUM)."""
            tok0 = b * TB + j * TH
            eng = nc.vector if j == 0 else nc.gpsimd
            xT = fw1.tile([128, NC1 * TH], F32, tag="fw1")
            u = fw1.tile([128, NC1 * TH], F32, tag="fw1")
            for g in range(4):
                xs = xinp.tile([128, 512], F32, tag="xin")
                nc.sync.dma_start(
                    xs[:], x_in[tok0:tok0 + TH, g * 512:(g + 1) * 512])
                pt = ps_x.tile([128, 512], F32, tag="x")
                for k in range(4):
                    nc.tensor.transpose(
                        pt[:, k * 128:(k + 1) * 128],
                        xs[:, k * 128:(k + 1) * 128], ident[:])
                nc.scalar.copy(xT[:, g * 512:(g + 1) * 512], pt[:])
            for g in range(4):
                u1 = ps_x.tile([128, 512], F32, tag="x")
                for pr in range(2):  # chunk pairs (4g+2pr, 4g+2pr+1)
                    c0, c1 = 4 * g + 2 * pr, 4 * g + 2 * pr + 1
                    sc = u1[:, (2 * pr) * TH:(2 * pr + 1) * TH]
                    dc = u1[:, (2 * pr + 1) * TH:(2 * pr + 2) * TH]
                    nc.tensor.matmul(sc, h128[:], xT[:, c0 * TH:(c0 + 1) * TH],
                                     start=True, stop=False)
                    nc.tensor.matmul(sc, h128[:], xT[:, c1 * TH:(c1 + 1) * TH],
                                     start=False, stop=True)
                    nc.tensor.matmul(dc, h128[:], xT[:, c0 * TH:(c0 + 1) * TH],
                                     start=True, stop=False)
                    nc.tensor.matmul(dc, negh[:], xT[:, c1 * TH:(c1 + 1) * TH],
                                     start=False, stop=True)
                nc.scalar.copy(u[:, g * 512:(g + 1) * 512], u1[:])
            _bfly_ip(eng, u, NC1, 2, TH, is_gp=(j == 1))
            _bfly_ip(eng, u, NC1, 4, TH, is_gp=(j == 1))
            _bfly_ip(eng, u, NC1, 8, TH, is_gp=(j == 1))
            # per-token absmax -> broadcast scales
            P1 = scl.tile([128, TH], F32, tag="p1")
            nc.vector.tensor_reduce(
                P1[:], u[:].rearrange("p (c t) -> p t c", c=NC1),
                mybir.AxisListType.X, MAX, apply_absolute_value=True)
            S1B = scl.tile([128, TH], F32, tag="s1b")
            nc.gpsimd.partition_all_reduce(S1B[:], P1[:], 128, RMAX)
            nc.vector.tensor_scalar(S1B[:], S1B[:], ISQ1, 1e-5, MULT, MAX)  # M1
            rM1 = scl.tile([128, TH], F32, tag="rm1")
            nc.vector.reciprocal(rM1[:], S1B[:])
            # cc slice for layer 2: cc = (M1*sW0)^2 / (127^2*64)
            ccs = ccw[:, j * TH:(j + 1) * TH]
            nc.vector.tensor_tensor(
                ccs, S1B[:], sW[:, 0:1].broadcast_to([128, TH]), MULT)
            nc.vector.tensor_tensor(ccs, ccs, ccs, MULT)
            nc.vector.tensor_scalar(
                ccs, ccs, 1.0 / (127.0 * 127.0 * 64.0), None, MULT)
            # s1t = rM1 * 127 * ISQ1 (in place); um = u * s1t ; q1 = round(um)
            nc.vector.tensor_scalar(rM1[:], rM1[:], 127.0 * ISQ1, None, MULT)
            uv = u[:].rearrange("p (c t) -> p c t", c=NC1)
            nc.vector.tensor_tensor(
                uv, uv, rM1[:, None, :].broadcast_to([128, NC1, TH]), MULT)
            q1v = q1[:].rearrange("p (c t) -> p c t", c=NC1)
            nc.vector.tensor_scalar(
                q1v[:, :, j * TH:(j + 1) * TH], uv, C_MAGIC, C_MAGIC, ADD, SUB)

        for b in range(NB):
            ccw = ccp.tile([128, TB], F32, tag="ccw")
            q1 = q1p.tile([128, NC1 * TB], BF16, tag="q1")
            x_half(b, 0, q1, ccw)
            x_half(b, 1, q1, ccw)
            if b == 0:
                for kc in range(NC1):
                    ternarize_t1(kc)

            # GEMM1 + relu^2 -> r (first fw2 tile)
            r = fw2.tile([128, NC2 * TB], F32, tag="fw2")
            for op_ in range(NC2 // 2):
                acc = ps_g1.tile([128, 512], F32, tag="a1")
                for half in range(2):
                    oc = 2 * op_ + half
                    for cp in range(NC1):
                        nc.tensor.matmul(
                            acc[:, half * TB:(half + 1) * TB],
                            t1[:, cp * I + oc * 128: cp * I + (oc + 1) * 128],
                            q1[:, cp * TB:(cp + 1) * TB],
                            start=(cp == 0), stop=(cp == NC1 - 1))
                nc.vector.tensor_scalar(acc[:], acc[:], 0.0, None, MAX)
                nc.scalar.activation(
                    r[:, op_ * 512:(op_ + 1) * 512], acc[:], AF.Square, bias=0.0)

            if b == 0:
                for mc in range(NC2):
                    ternarize_t2(mc)

            # FWHT2: H128 per chunk-pair + stage-1 from PSUM, then in-place stages
            for g in range(NC2 // 2):
                v1 = ps_h2.tile([128, 512], F32, tag="h2")
                m0, m1 = 2 * g, 2 * g + 1
                sc = v1[:, 0:TB]
                dc = v1[:, TB:2 * TB]
                nc.tensor.matmul(sc, h128[:], r[:, m0 * TB:(m0 + 1) * TB],
                                 start=True, stop=False)
                nc.tensor.matmul(sc, h128[:], r[:, m1 * TB:(m1 + 1) * TB],
                                 start=False, stop=True)
                nc.tensor.matmul(dc, h128[:], r[:, m0 * TB:(m0 + 1) * TB],
                                 start=True, stop=False)
                nc.tensor.matmul(dc, negh[:], r[:, m1 * TB:(m1 + 1) * TB],
                                 start=False, stop=True)
                nc.scalar.copy(r[:, g * 512:(g + 1) * 512], v1[:])
            FR = 0.6  # DVE share of stages 2-5
            for sg in (2, 4, 8, 16):
                _bfly_ip(nc.vector, r, NC2, sg, TB, 0.0, FR)
                _bfly_ip(nc.gpsimd, r, NC2, sg, TB, FR, 1.0, is_gp=True)
            # layer-2 scales
            P2 = scl.tile([128, TB], F32, tag="p2")
            nc.vector.tensor_reduce(
                P2[:], r[:].rearrange("p (m t) -> p t m", m=NC2),
                mybir.AxisListType.X, MAX, apply_absolute_value=True)
            S2B = scl.tile([128, TB], F32, tag="s2b")
            nc.gpsimd.partition_all_reduce(S2B[:], P2[:], 128, RMAX)
            nc.vector.tensor_tensor(S2B[:], S2B[:], ccw[:], MULT)       # M2
            nc.vector.tensor_scalar(S2B[:], S2B[:], 1e-5, None, MAX)
            rM2 = scl.tile([128, TB], F32, tag="rm2")
            nc.vector.reciprocal(rM2[:], S2B[:])
            s2t = scl.tile([128, TB], F32, tag="s2t")
            nc.vector.tensor_tensor(s2t[:], rM2[:], ccw[:], MULT)
            nc.vector.tensor_scalar(s2t[:], s2t[:], 127.0, None, MULT)
            # fb = M2 * sW1 / 127 (in place over S2B/M2)
            nc.vector.tensor_tensor(
                S2B[:], S2B[:], sW[:, 1:2].broadcast_to([128, TB]), MULT)
            nc.vector.tensor_scalar(S2B[:], S2B[:], 1.0 / 127.0, None, MULT)
            fcols = []
            for j in range(2):
                fps = ps_x.tile([128, 512], F32, tag="x")
                nc.tensor.transpose(fps[:, 0:128], S2B[:, j * TH:(j + 1) * TH],
                                    ident[:])
                fcol = tiny.tile([128, 1], F32, tag=f"fcol{j}")
                nc.vector.tensor_copy(fcol[:], fps[:, 0:1])
                fcols.append(fcol)
            # vm in place (split V/G), then q2 round
            rv = r[:].rearrange("p (m t) -> p m t", m=NC2)
            s2b = s2t[:, None, :].broadcast_to([128, NC2, TB])
            MS = int(NC2 * 0.6)
            nc.vector.tensor_tensor(rv[:, 0:MS, :], rv[:, 0:MS, :],
                                    s2b[:, 0:MS, :], MULT)
            nc.gpsimd.tensor_tensor(rv[:, MS:NC2, :], rv[:, MS:NC2, :],
                                    s2b[:, MS:NC2, :], MULT)
            q2 = q2p.tile([128, NC2 * TB], BF16, tag="q2")
            nc.vector.tensor_scalar(q2[:], r[:], C_MAGIC, C_MAGIC, ADD, SUB)

            # GEMM2 tokens-stationary; t2 eighth-slabs restreamed per token-half
            for j in range(2):
                acc2s = []
                for hs in range(4):
                    a2t = ps_g2.tile([128, 512], F32, tag="a2")
                    acc2s.append(a2t)
                for e in range(16):
                    st = slab.tile([128, 2 * H], FP8, tag="t2s")
                    nc.sync.dma_start(
                        st[:].rearrange("p (m h) -> p m h", m=2),
                        t2_dram[e // 4][(e % 4) * 256:(e % 4) * 256 + 256, :]
                        .rearrange("(m p) h -> p m h", p=128))
                    for hs in range(4):
                        for mi in range(2):
                            m2 = e * 2 + mi
                            nc.tensor.matmul(
                                acc2s[hs][:],
                                q2[:, m2 * TB + j * TH: m2 * TB + (j + 1) * TH],
                                st[:, mi * H + hs * 512: mi * H + (hs + 1) * 512],
                                start=(m2 == 0), stop=(m2 == NC2 - 1))
                for hs in range(4):
                    ot = otp.tile([128, 512], F32, tag="ot")
                    nc.scalar.activation(ot[:], acc2s[hs][:], AF.Identity,
                                         bias=0.0, scale=fcols[j][:])
                    nc.sync.dma_start(
                        out_d[b * TB + j * TH: b * TB + (j + 1) * TH,
                              hs * 512:(hs + 1) * 512], ot[:])

    nc.finalize()
    return nc


_NC_CACHE = None


def _get_nc():
    global _NC_CACHE
    if _NC_CACHE is None:
        _NC_CACHE = build()
    return _NC_CACHE


def _hadamard128():
    h = np.array([[1.0]], dtype=np.float32)
    while h.shape[0] < 128:
        h = np.block([[h, h], [h, -h]])
    return h.astype(np.float32)


def kernel(hidden_states, w_up, w_down):
    x = np.ascontiguousarray(hidden_states.reshape(TOKENS, H), dtype=np.float32)
    wuT = np.ascontiguousarray(w_up.T, dtype=np.float32)
    wdT = np.ascontiguousarray(w_down.T, dtype=np.float32)
    h128 = _hadamard128()

    nc = _get_nc()
    in_maps = []
    for c in range(NCORES):
        in_maps.append({
            "x": x[c * T:(c + 1) * T],
            "wuT": wuT,
            "wdT": wdT,
            "wu_s": wuT[c * (H // NCORES):(c + 1) * (H // NCORES)],
            "wd_s": wdT[c * (I // NCORES):(c + 1) * (I // NCORES)],
            "h128": h128,
        })
    res = run_bass_kernel_spmd(nc, in_maps, list(range(NCORES))).results
    out = np.concatenate(
        [np.asarray(res[c]["out"], dtype=np.float32) for c in range(NCORES)], axis=0
    )
    return out.reshape(B, S, H)



# revision 13
# speedup vs baseline: 1.6527x; 1.6527x over previous
"""BitFeedForward (Hadamard + int8 act-quant + ternary weights) on 8 TRN2 cores.

v3 — data-parallel over tokens (8192 -> 1024/core), restructured from v2:
  - weight ternarization sharded 8x across cores (each core ternarizes 1/8
    of w_up and w_down), then fp8 AllGather; t1 resident in SBUF, t2
    streamed once per block from the gathered DRAM buffer.
  - per-token absmax via PE-transpose + free-dim reduce + K=1 matmul
    broadcast (no gpsimd partition_all_reduce on the hot path).
  - plain H128 matmul stages (single stationary operand) + in-place
    DVE/GPSIMD butterflies for all inter-chunk FWHT stages.
  - relu^2 evacuation fused into one DVE scalar_tensor_tensor per slice.
  - 2-deep software-pipelined emission so the PE stream
    [gemm2(k-2) | xA(k+1) | gemm1(k) | h128_2(k)] never waits on the DVE
    butterfly chain.
"""
import math
import numpy as np
from contextlib import ExitStack

import concourse.bass as bass
from concourse import bacc
import concourse.tile as tile
import concourse.mybir as mybir
from concourse.bass_utils import run_bass_kernel_spmd
from concourse.masks import make_identity

F32 = mybir.dt.float32
BF16 = mybir.dt.bfloat16
FP8 = mybir.dt.float8e4

NCORES = 8
B, S, H, I = 4, 2048, 2048, 4096
TOKENS = B * S
T = TOKENS // NCORES
TB = 256
NB = T // TB
TH = 128
NC1 = H // 128
NC2 = I // 128
C_MAGIC = 12582912.0
ISQ1 = 1.0 / math.sqrt(H)
WCOUNT = float(H * I)
FR1 = 0.75  # DVE share of FWHT1 butterflies
FR2 = 0.70  # DVE share of FWHT2 butterflies
MSF = 0.72  # DVE share of the scale-multiplies

ADD = mybir.AluOpType.add
SUB = mybir.AluOpType.subtract
MULT = mybir.AluOpType.mult
MAX = mybir.AluOpType.max
BYPASS = mybir.AluOpType.bypass
AF = mybir.ActivationFunctionType
AX = mybir.AxisListType.X


def _bfly_ip(eng, t, nchunk, sigma, span, lo=0.0, hi=1.0, is_gp=False):
    """In-place butterfly over the chunk axis: a' = a+b ; b' = a' - 2b.

    t: tile [128, nchunk*span]; chunk c at cols [c*span,(c+1)*span).
    lo/hi select a fraction of the s-subrange for engine splitting.
    """
    v = t[:].rearrange("p (g two s) -> p g two s", two=2, s=sigma * span)
    n = sigma * span
    s0, s1 = int(n * lo), int(n * hi)
    if s1 <= s0:
        return
    a = v[:, :, 0, s0:s1]
    bv = v[:, :, 1, s0:s1]
    eng.tensor_tensor(a, a, bv, ADD)
    if is_gp:
        eng.tensor_tensor(bv, bv, bv, ADD)
        eng.tensor_tensor(bv, a, bv, SUB)
    else:
        eng.scalar_tensor_tensor(bv, bv, -2.0, a, MULT, ADD)


def build():
    nc = bacc.Bacc()
    x_in = nc.declare_dram_parameter("x", [T, H], F32, isOutput=False)
    wus_in = nc.declare_dram_parameter("wu_s", [H // NCORES, I], F32, isOutput=False)
    wds_in = nc.declare_dram_parameter("wd_s", [I // NCORES, H], F32, isOutput=False)
    h128_in = nc.declare_dram_parameter("h128", [128, 128], F32, isOutput=False)
    out_d = nc.declare_dram_parameter("out", [T, H], F32, isOutput=True)

    RG = [list(range(NCORES))]

    with tile.TileContext(nc) as tc, ExitStack() as ctx:
        const = ctx.enter_context(tc.tile_pool(name="const", bufs=1))
        wq = ctx.enter_context(tc.tile_pool(name="wq", bufs=2))
        tsb = ctx.enter_context(tc.tile_pool(name="tsb", bufs=2))
        t1p = ctx.enter_context(tc.tile_pool(name="t1", bufs=1))
        slab = ctx.enter_context(tc.tile_pool(name="slab", bufs=3))
        xinp = ctx.enter_context(tc.tile_pool(name="xinp", bufs=3))
        fw1 = ctx.enter_context(tc.tile_pool(name="fw1", bufs=2))
        q1p = ctx.enter_context(tc.tile_pool(name="q1", bufs=2))
        rp = ctx.enter_context(tc.tile_pool(name="rp", bufs=2))
        q2p = ctx.enter_context(tc.tile_pool(name="q2", bufs=2))
        otp = ctx.enter_context(tc.tile_pool(name="otp", bufs=2))
        scl = ctx.enter_context(tc.tile_pool(name="scl", bufs=2))
        tiny = ctx.enter_context(tc.tile_pool(name="tiny", bufs=6))
        dram = ctx.enter_context(tc.tile_pool(name="dram", bufs=1, space="DRAM"))
        ps_m = ctx.enter_context(tc.tile_pool(name="ps_m", bufs=2, space="PSUM"))
        ps_g1 = ctx.enter_context(tc.tile_pool(name="ps_g1", bufs=2, space="PSUM"))
        ps_g2 = ctx.enter_context(tc.tile_pool(name="ps_g2", bufs=4, space="PSUM"))

        ident = const.tile([128, 128], F32)
        make_identity(nc, ident[:])
        h128 = const.tile([128, 128], F32)
        nc.sync.dma_start(h128[:], h128_in[:])
        ones_col = const.tile([128, 1], F32)
        nc.vector.memset(ones_col[:], 1.0)
        ones_row = const.tile([1, 128], F32)
        nc.vector.memset(ones_row[:], 1.0)
        biasC = const.tile([128, 1], F32)
        nc.vector.memset(biasC[:], C_MAGIC)
        biasNC = const.tile([128, 1], F32)
        nc.vector.memset(biasNC[:], -C_MAGIC)

        # ------------- weight scale partials + AllReduce -------------
        def wchunks(src, nrow, width):
            """Yield ([128,2048] slice-view, row, colhalf) subchunks."""
            for i in range(nrow):
                for hc in range(width // 2048):
                    yield (src[i * 128:(i + 1) * 128,
                               hc * 2048:(hc + 1) * 2048], i, hc)

        def abs_total(src, nrow, width, tagp):
            parts = []
            for sl, i, hc in wchunks(src, nrow, width):
                ch = wq.tile([128, 2048], F32, tag="wch", name=f"w_{tagp}_{i}_{hc}")
                nc.gpsimd.dma_start(ch[:], sl)
                acc = tiny.tile([128, 1], F32, tag="wacc")
                nc.vector.tensor_reduce(acc[:], ch[:], AX, ADD,
                                        apply_absolute_value=True)
                parts.append(acc)
            tot = tiny.tile([128, 1], F32, tag=f"{tagp}tot")
            nc.vector.tensor_add(tot[:], parts[0][:], parts[1][:])
            for a in parts[2:]:
                nc.vector.tensor_add(tot[:], tot[:], a[:])
            return tot

        pu = abs_total(wus_in, 2, I, "au")
        pd = abs_total(wds_in, 4, H, "ad")
        psum2 = ps_m.tile([1, 2], F32, tag="pm")
        nc.tensor.matmul(psum2[:, 0:1], pu[:], ones_col[:], start=True, stop=True)
        nc.tensor.matmul(psum2[:, 1:2], pd[:], ones_col[:], start=True, stop=True)
        part = tiny.tile([1, 2], F32)
        nc.vector.tensor_copy(part[:], psum2[:])

        ccin = dram.tile([1, 2], F32, tag="ccin")
        ccout = dram.tile([1, 2], F32, tag="ccout", addr_space="Shared")
        nc.gpsimd.dma_start(ccin[:], part[:])
        nc.gpsimd.collective_compute(
            "AllReduce", ADD, replica_groups=RG,
            ins=[ccin.opt()], outs=[ccout.opt()])
        res2 = tiny.tile([1, 2], F32)
        nc.sync.dma_start(res2[:], ccout[:])

        srow = tiny.tile([1, 2], F32)
        nc.vector.tensor_scalar(srow[:], res2[:], 1.0 / WCOUNT, 1e-5, MULT, MAX)
        sW_ps = ps_m.tile([128, 2], F32, tag="pm")
        nc.tensor.matmul(sW_ps[:], ones_row[:], srow[:], start=True, stop=True)
        sW = tiny.tile([128, 2], F32)
        nc.vector.tensor_copy(sW[:], sW_ps[:])
        rW = tiny.tile([128, 2], F32)
        nc.vector.reciprocal(rW[:], sW[:])

        # ------------- sharded ternarize + fp8 AllGather -------------
        t1s = dram.tile([H // NCORES, I], FP8, tag="t1s")
        t1g = dram.tile([H, I], FP8, tag="t1g", addr_space="Shared")
        t2s = dram.tile([I // NCORES, H], FP8, tag="t2s")
        t2g = dram.tile([I, H], FP8, tag="t2g", addr_space="Shared")

        def ternarize(src, nrow, width, col, dst, tagp):
            for sl, i, hc in wchunks(src, nrow, width):
                ch = wq.tile([128, 2048], F32, tag="wch", name=f"wb_{tagp}_{i}_{hc}")
                nc.gpsimd.dma_start(ch[:], sl)
                ob = tsb.tile([128, 2048], FP8, tag="tsb")
                for q in range(4):
                    tmp = ps_m.tile([128, 512], F32, tag="pm")
                    nc.scalar.activation(tmp[:], ch[:, q * 512:(q + 1) * 512],
                                         AF.Identity, bias=biasC[:],
                                         scale=rW[:, col:col + 1])
                    nc.scalar.activation(ob[:, q * 512:(q + 1) * 512], tmp[:],
                                         AF.Sign, bias=biasNC[:], scale=1.0)
                nc.sync.dma_start(dst[i * 128:(i + 1) * 128,
                                      hc * 2048:(hc + 1) * 2048], ob[:])

        ternarize(wus_in, 2, I, 0, t1s, "u")
        ternarize(wds_in, 4, H, 1, t2s, "d")
        nc.gpsimd.collective_compute(
            "AllGather", BYPASS, replica_groups=RG,
            ins=[t1s.opt()], outs=[t1g.opt()])
        nc.gpsimd.collective_compute(
            "AllGather", BYPASS, replica_groups=RG,
            ins=[t2s.opt()], outs=[t2g.opt()])

        t1 = t1p.tile([128, NC1 * I], FP8)  # resident ternary w_up^T
        for kc in range(NC1):
            nc.sync.dma_start(t1[:, kc * I:(kc + 1) * I],
                              t1g[kc * 128:(kc + 1) * 128, :])

        # ------------- per-token scale helpers -------------
        def col_scales_bcast(vec):
            """[128,1] token-on-partition vector -> [128, TH] SBUF broadcast."""
            rps = ps_m.tile([1, 128], F32, tag="pm")
            nc.tensor.transpose(rps[:], vec[:], ident[:])
            row = tiny.tile([1, 128], F32, tag="row")
            nc.vector.tensor_copy(row[:], rps[:])
            bps = ps_m.tile([128, 128], F32, tag="pm")
            nc.tensor.matmul(bps[:], ones_row[:], row[:], start=True, stop=True)
            sb = scl.tile([128, TH], F32, tag="bc")
            nc.scalar.copy(sb[:], bps[:])
            return sb

        def x_half(b, j, q1):
            """Transpose + FWHT + int8 quant for token-half j of block b.

            Returns the per-token M1 vector (kept for layer-2 scales)."""
            tok0 = b * TB + j * TH
            u = fw1.tile([128, NC1 * TH], F32, tag="fw1")
            for g in range(4):
                xs = xinp.tile([128, 512], F32, tag="xin")
                nc.sync.dma_start(
                    xs[:], x_in[tok0:tok0 + TH, g * 512:(g + 1) * 512])
                pt = ps_m.tile([128, 512], F32, tag="pm")
                for k in range(4):
                    nc.tensor.transpose(
                        pt[:, k * 128:(k + 1) * 128],
                        xs[:, k * 128:(k + 1) * 128], ident[:])
                nc.scalar.copy(u[:, g * 512:(g + 1) * 512], pt[:])
            for g in range(4):
                u1 = ps_m.tile([128, 512], F32, tag="pm")
                for k in range(4):
                    c = 4 * g + k
                    nc.tensor.matmul(u1[:, k * TH:(k + 1) * TH], h128[:],
                                     u[:, c * TH:(c + 1) * TH],
                                     start=True, stop=True)
                nc.scalar.copy(u[:, g * 512:(g + 1) * 512], u1[:])
            for sg in (1, 2, 4, 8):
                _bfly_ip(nc.vector, u, NC1, sg, TH, 0.0, FR1)
                _bfly_ip(nc.gpsimd, u, NC1, sg, TH, FR1, 1.0, is_gp=True)
            # per-token absmax over [partitions x chunks]
            P1 = scl.tile([128, TH], F32, tag="p1")
            nc.vector.tensor_reduce(
                P1[:], u[:].rearrange("p (c t) -> p t c", c=NC1),
                AX, MAX, apply_absolute_value=True)
            tps = ps_m.tile([128, 128], F32, tag="pm")
            nc.tensor.transpose(tps[:], P1[:], ident[:])
            M1 = tiny.tile([128, 1], F32, tag="m1")
            nc.vector.tensor_reduce(M1[:], tps[:], AX, MAX,
                                    apply_absolute_value=True)
            nc.vector.tensor_scalar(M1[:], M1[:], ISQ1, 1e-5, MULT, MAX)
            s1t = tiny.tile([128, 1], F32, tag="s1t")
            nc.vector.reciprocal(s1t[:], M1[:])
            nc.vector.tensor_scalar(s1t[:], s1t[:], 127.0 * ISQ1, None, MULT)
            s1b = col_scales_bcast(s1t)
            uv = u[:].rearrange("p (c t) -> p c t", c=NC1)
            sbb = s1b[:, None, :].broadcast_to([128, NC1, TH])
            MS = int(NC1 * MSF)
            nc.vector.tensor_tensor(uv[:, 0:MS, :], uv[:, 0:MS, :],
                                    sbb[:, 0:MS, :], MULT)
            nc.gpsimd.tensor_tensor(uv[:, MS:NC1, :], uv[:, MS:NC1, :],
                                    sbb[:, MS:NC1, :], MULT)
            q1v = q1[:].rearrange("p (c t) -> p c t", c=NC1)
            nc.vector.tensor_scalar(
                q1v[:, :, j * TH:(j + 1) * TH], uv, C_MAGIC, C_MAGIC, ADD, SUB)
            return M1

        def gemm1(q1, rjs):
            """GEMM1 + fused relu^2 evac into per-half r tiles rjs[j]."""
            for op_ in range(NC2 // 2):
                acc = ps_g1.tile([128, 512], F32, tag="a1")
                for half in range(2):
                    oc = 2 * op_ + half
                    for cp in range(NC1):
                        nc.tensor.matmul(
                            acc[:, half * TB:(half + 1) * TB],
                            t1[:, cp * I + oc * 128: cp * I + (oc + 1) * 128],
                            q1[:, cp * TB:(cp + 1) * TB],
                            start=(cp == 0), stop=(cp == NC1 - 1))
                av = acc[:].rearrange("p (o t) -> p o t", o=2)
                for j in range(2):
                    rv = rjs[j][:].rearrange("p (m t) -> p m t", m=NC2)
                    sl = av[:, :, j * TH:(j + 1) * TH]
                    dst = rv[:, 2 * op_:2 * op_ + 2, :]
                    nc.vector.tensor_scalar(dst, sl, 0.0, None, MAX)
                    nc.scalar.activation(dst, dst, AF.Square, bias=0.0)

        def h128_2(rj):
            for g in range(NC2 // 4):
                ps = ps_m.tile([128, 512], F32, tag="pm")
                for k in range(4):
                    m = 4 * g + k
                    nc.tensor.matmul(ps[:, k * TH:(k + 1) * TH], h128[:],
                                     rj[:, m * TH:(m + 1) * TH],
                                     start=True, stop=True)
                nc.scalar.copy(rj[:, g * 512:(g + 1) * 512], ps[:])

        def quant2(rj, M1, j, q2):
            for sg in (1, 2, 4, 8, 16):
                _bfly_ip(nc.vector, rj, NC2, sg, TH, 0.0, FR2)
                _bfly_ip(nc.gpsimd, rj, NC2, sg, TH, FR2, 1.0, is_gp=True)
            # cc = (M1*sW0)^2 / (127^2 * 64)
            cc = tiny.tile([128, 1], F32, tag="cc")
            nc.vector.tensor_tensor(cc[:], M1[:], sW[:, 0:1], MULT)
            nc.vector.tensor_tensor(cc[:], cc[:], cc[:], MULT)
            nc.vector.tensor_scalar(cc[:], cc[:], 1.0 / (127.0 * 127.0 * 64.0),
                                    None, MULT)
            P2 = scl.tile([128, TH], F32, tag="p2")
            nc.vector.tensor_reduce(
                P2[:], rj[:].rearrange("p (m t) -> p t m", m=NC2),
                AX, MAX, apply_absolute_value=True)
            tps = ps_m.tile([128, 128], F32, tag="pm")
            nc.tensor.transpose(tps[:], P2[:], ident[:])
            M2 = tiny.tile([128, 1], F32, tag="m2")
            nc.vector.tensor_reduce(M2[:], tps[:], AX, MAX,
                                    apply_absolute_value=True)
            nc.vector.tensor_tensor(M2[:], M2[:], cc[:], MULT)
            nc.vector.tensor_scalar(M2[:], M2[:], 1e-5, None, MAX)
            s2t = tiny.tile([128, 1], F32, tag="s2t")
            nc.vector.reciprocal(s2t[:], M2[:])
            nc.vector.tensor_tensor(s2t[:], s2t[:], cc[:], MULT)
            nc.vector.tensor_scalar(s2t[:], s2t[:], 127.0, None, MULT)
            fb = tiny.tile([128, 1], F32, tag=f"fb{j}")
            nc.vector.tensor_tensor(fb[:], M2[:], sW[:, 1:2], MULT)
            nc.vector.tensor_scalar(fb[:], fb[:], 1.0 / 127.0, None, MULT)
            s2b = col_scales_bcast(s2t)
            rv = rj[:].rearrange("p (m t) -> p m t", m=NC2)
            sbb = s2b[:, None, :].broadcast_to([128, NC2, TH])
            MS = int(NC2 * MSF)
            nc.vector.tensor_tensor(rv[:, 0:MS, :], rv[:, 0:MS, :],
                                    sbb[:, 0:MS, :], MULT)
            nc.gpsimd.tensor_tensor(rv[:, MS:NC2, :], rv[:, MS:NC2, :],
                                    sbb[:, MS:NC2, :], MULT)
            nc.vector.tensor_scalar(q2[:], rj[:], C_MAGIC, C_MAGIC, ADD, SUB)
            return fb

        def gemm2(b, j, q2, fb):
            acc2s = [ps_g2.tile([128, 512], F32, tag="a2", name=f"a2_{hs}")
                     for hs in range(4)]
            for e in range(NC2 // 2):
                st = slab.tile([128, 2 * H], FP8, tag="t2s")
                nc.sync.dma_start(
                    st[:].rearrange("p (m h) -> p m h", m=2),
                    t2g[e * 256:(e + 1) * 256, :]
                    .rearrange("(m p) h -> p m h", p=128))
                for hs in range(4):
                    for mi in range(2):
                        m2 = e * 2 + mi
                        nc.tensor.matmul(
                            acc2s[hs][:],
                            q2[:, m2 * TH:(m2 + 1) * TH],
                            st[:, mi * H + hs * 512: mi * H + (hs + 1) * 512],
                            start=(m2 == 0), stop=(m2 == NC2 - 1))
            for hs in range(4):
                ot = otp.tile([128, 512], F32, tag="ot")
                nc.scalar.activation(ot[:], acc2s[hs][:], AF.Identity,
                                     bias=0.0, scale=fb[:])
                nc.sync.dma_start(
                    out_d[b * TB + j * TH: b * TB + (j + 1) * TH,
                          hs * 512:(hs + 1) * 512], ot[:])

        # ------------- 2-deep software-pipelined block loop -------------
        # iteration k PE stream: gemm2(k-2) | xA(k+1) | gemm1(k) | h128_2(k)
        # iteration k DVE stream: bf2+quant2(k-1) | bf1+quant1(k+1) | evac(k)
        q1s = {}
        rs = {}
        m1s = {}
        q2s = {}
        fbs = {}
        q1s[0] = q1p.tile([128, NC1 * TB], BF16, tag="q1", name="q1_0")
        m1s[0] = [x_half(0, j, q1s[0]) for j in range(2)]
        for k in range(NB + 2):
            if 0 <= k - 2 < NB:
                for j in range(2):
                    gemm2(k - 2, j, q2s[(k - 2, j)], fbs[(k - 2, j)])
                    del q2s[(k - 2, j)], fbs[(k - 2, j)]
            if 0 <= k - 1 < NB:
                for j in range(2):
                    q2 = q2p.tile([128, NC2 * TH], BF16, tag="q2",
                                  name=f"q2_{k - 1}_{j}")
                    fbs[(k - 1, j)] = quant2(rs[k - 1][j], m1s[k - 1][j], j, q2)
                    q2s[(k - 1, j)] = q2
                del rs[k - 1], m1s[k - 1]
            if k + 1 < NB:
                q1s[k + 1] = q1p.tile([128, NC1 * TB], BF16, tag="q1",
                                      name=f"q1_{k + 1}")
                m1s[k + 1] = [x_half(k + 1, j, q1s[k + 1]) for j in range(2)]
            if k < NB:
                rjs = [rp.tile([128, NC2 * TH], F32, tag="rj",
                               name=f"r_{k}_{j}") for j in range(2)]
                gemm1(q1s[k], rjs)
                del q1s[k]
                h128_2(rjs[0])
                h128_2(rjs[1])
                rs[k] = rjs

    nc.finalize()
    return nc


_NC_CACHE = None


def _get_nc():
    global _NC_CACHE
    if _NC_CACHE is None:
        _NC_CACHE = build()
    return _NC_CACHE


def _hadamard128():
    h = np.array([[1.0]], dtype=np.float32)
    while h.shape[0] < 128:
        h = np.block([[h, h], [h, -h]])
    return h.astype(np.float32)


def kernel(hidden_states, w_up, w_down):
    x = np.ascontiguousarray(hidden_states.reshape(TOKENS, H), dtype=np.float32)
    wuT = np.ascontiguousarray(w_up.T, dtype=np.float32)
    wdT = np.ascontiguousarray(w_down.T, dtype=np.float32)
    h128 = _hadamard128()

    nc = _get_nc()
    in_maps = []
    for c in range(NCORES):
        in_maps.append({
            "x": x[c * T:(c + 1) * T],
            "wu_s": np.ascontiguousarray(
                wuT[c * (H // NCORES):(c + 1) * (H // NCORES)]),
            "wd_s": np.ascontiguousarray(
                wdT[c * (I // NCORES):(c + 1) * (I // NCORES)]),
            "h128": h128,
        })
    res = run_bass_kernel_spmd(nc, in_maps, list(range(NCORES))).results
    out = np.concatenate(
        [np.asarray(res[c]["out"], dtype=np.float32) for c in range(NCORES)], axis=0
    )
    return out.reshape(B, S, H)


# revision 14
# speedup vs baseline: 1.7293x; 1.0463x over previous
"""BitFeedForward (Hadamard + int8 act-quant + ternary weights) on 8 TRN2 cores.

v3 — data-parallel over tokens (8192 -> 1024/core), restructured from v2:
  - weight ternarization sharded 8x across cores (each core ternarizes 1/8
    of w_up and w_down), then fp8 AllGather; t1 resident in SBUF, t2
    streamed once per block from the gathered DRAM buffer.
  - per-token absmax via PE-transpose + free-dim reduce + K=1 matmul
    broadcast (no gpsimd partition_all_reduce on the hot path).
  - plain H128 matmul stages (single stationary operand) + in-place
    DVE/GPSIMD butterflies for all inter-chunk FWHT stages.
  - relu^2 evacuation fused into one DVE scalar_tensor_tensor per slice.
  - 2-deep software-pipelined emission so the PE stream
    [gemm2(k-2) | xA(k+1) | gemm1(k) | h128_2(k)] never waits on the DVE
    butterfly chain.
"""
import math
import numpy as np
from contextlib import ExitStack

import concourse.bass as bass
from concourse import bacc
import concourse.tile as tile
import concourse.mybir as mybir
from concourse.bass_utils import run_bass_kernel_spmd
from concourse.masks import make_identity

F32 = mybir.dt.float32
BF16 = mybir.dt.bfloat16
FP8 = mybir.dt.float8e4

NCORES = 8
B, S, H, I = 4, 2048, 2048, 4096
TOKENS = B * S
T = TOKENS // NCORES
TB = 256
NB = T // TB
TH = 128
NC1 = H // 128
NC2 = I // 128
C_MAGIC = 12582912.0
ISQ1 = 1.0 / math.sqrt(H)
WCOUNT = float(H * I)
FR1 = 0.80  # DVE share of FWHT1 butterflies
FR2 = 0.78  # DVE share of FWHT2 butterflies
MSF = 0.78  # DVE share of the scale-multiplies

ADD = mybir.AluOpType.add
SUB = mybir.AluOpType.subtract
MULT = mybir.AluOpType.mult
MAX = mybir.AluOpType.max
MIN = mybir.AluOpType.min
BYPASS = mybir.AluOpType.bypass
AF = mybir.ActivationFunctionType
AX = mybir.AxisListType.X


def _bfly_ip(eng, t, nchunk, sigma, span, lo=0.0, hi=1.0, is_gp=False):
    """In-place butterfly over the chunk axis: a' = a+b ; b' = a' - 2b.

    t: tile [128, nchunk*span]; chunk c at cols [c*span,(c+1)*span).
    lo/hi select a fraction of the s-subrange for engine splitting.
    """
    v = t[:].rearrange("p (g two s) -> p g two s", two=2, s=sigma * span)
    n = sigma * span
    s0, s1 = int(n * lo), int(n * hi)
    if s1 <= s0:
        return
    a = v[:, :, 0, s0:s1]
    bv = v[:, :, 1, s0:s1]
    eng.tensor_tensor(a, a, bv, ADD)
    if is_gp:
        eng.tensor_tensor(bv, bv, bv, ADD)
        eng.tensor_tensor(bv, a, bv, SUB)
    else:
        eng.scalar_tensor_tensor(bv, bv, -2.0, a, MULT, ADD)


def build():
    nc = bacc.Bacc()
    x_in = nc.declare_dram_parameter("x", [T, H], F32, isOutput=False)
    wus_in = nc.declare_dram_parameter("wu_s", [H // NCORES, I], F32, isOutput=False)
    wds_in = nc.declare_dram_parameter("wd_s", [I // NCORES, H], F32, isOutput=False)
    h128_in = nc.declare_dram_parameter("h128", [128, 128], F32, isOutput=False)
    out_d = nc.declare_dram_parameter("out", [T, H], F32, isOutput=True)

    RG = [list(range(NCORES))]

    with tile.TileContext(nc) as tc, ExitStack() as ctx:
        const = ctx.enter_context(tc.tile_pool(name="const", bufs=1))
        wq = ctx.enter_context(tc.tile_pool(name="wq", bufs=2))
        tsb = ctx.enter_context(tc.tile_pool(name="tsb", bufs=2))
        t1p = ctx.enter_context(tc.tile_pool(name="t1", bufs=1))
        slab = ctx.enter_context(tc.tile_pool(name="slab", bufs=3))
        xinp = ctx.enter_context(tc.tile_pool(name="xinp", bufs=3))
        fw1 = ctx.enter_context(tc.tile_pool(name="fw1", bufs=2))
        q1p = ctx.enter_context(tc.tile_pool(name="q1", bufs=2))
        rp = ctx.enter_context(tc.tile_pool(name="rp", bufs=2))
        q2p = ctx.enter_context(tc.tile_pool(name="q2", bufs=2))
        otp = ctx.enter_context(tc.tile_pool(name="otp", bufs=2))
        scl = ctx.enter_context(tc.tile_pool(name="scl", bufs=2))
        tiny = ctx.enter_context(tc.tile_pool(name="tiny", bufs=6))
        dram = ctx.enter_context(tc.tile_pool(name="dram", bufs=1, space="DRAM"))
        ps_m = ctx.enter_context(tc.tile_pool(name="ps_m", bufs=2, space="PSUM"))
        ps_g1 = ctx.enter_context(tc.tile_pool(name="ps_g1", bufs=2, space="PSUM"))
        ps_g2 = ctx.enter_context(tc.tile_pool(name="ps_g2", bufs=4, space="PSUM"))

        ident = const.tile([128, 128], F32)
        make_identity(nc, ident[:])
        h128 = const.tile([128, 128], F32)
        nc.sync.dma_start(h128[:], h128_in[:])
        ones_col = const.tile([128, 1], F32)
        nc.vector.memset(ones_col[:], 1.0)
        ones_row = const.tile([1, 128], F32)
        nc.vector.memset(ones_row[:], 1.0)
        biasC = const.tile([128, 1], F32)
        nc.vector.memset(biasC[:], C_MAGIC)
        biasNC = const.tile([128, 1], F32)
        nc.vector.memset(biasNC[:], -C_MAGIC)

        # ------------- weight scale partials + AllReduce -------------
        def wchunks(src, nrow, width):
            """Yield ([128,2048] slice-view, row, colhalf) subchunks."""
            for i in range(nrow):
                for hc in range(width // 2048):
                    yield (src[i * 128:(i + 1) * 128,
                               hc * 2048:(hc + 1) * 2048], i, hc)

        def abs_total(src, nrow, width, tagp):
            parts = []
            for sl, i, hc in wchunks(src, nrow, width):
                ch = wq.tile([128, 2048], F32, tag="wch", name=f"w_{tagp}_{i}_{hc}")
                nc.gpsimd.dma_start(ch[:], sl)
                acc = tiny.tile([128, 1], F32, tag="wacc")
                nc.vector.tensor_reduce(acc[:], ch[:], AX, ADD,
                                        apply_absolute_value=True)
                parts.append(acc)
            tot = tiny.tile([128, 1], F32, tag=f"{tagp}tot")
            nc.vector.tensor_add(tot[:], parts[0][:], parts[1][:])
            for a in parts[2:]:
                nc.vector.tensor_add(tot[:], tot[:], a[:])
            return tot

        pu = abs_total(wus_in, 2, I, "au")
        pd = abs_total(wds_in, 4, H, "ad")
        psum2 = ps_m.tile([1, 2], F32, tag="pm")
        nc.tensor.matmul(psum2[:, 0:1], pu[:], ones_col[:], start=True, stop=True)
        nc.tensor.matmul(psum2[:, 1:2], pd[:], ones_col[:], start=True, stop=True)
        part = tiny.tile([1, 2], F32)
        nc.vector.tensor_copy(part[:], psum2[:])

        ccin = dram.tile([1, 2], F32, tag="ccin")
        ccout = dram.tile([1, 2], F32, tag="ccout", addr_space="Shared")
        nc.gpsimd.dma_start(ccin[:], part[:])
        nc.gpsimd.collective_compute(
            "AllReduce", ADD, replica_groups=RG,
            ins=[ccin.opt()], outs=[ccout.opt()])
        res2 = tiny.tile([1, 2], F32)
        nc.sync.dma_start(res2[:], ccout[:])

        srow = tiny.tile([1, 2], F32)
        nc.vector.tensor_scalar(srow[:], res2[:], 1.0 / WCOUNT, 1e-5, MULT, MAX)
        sW_ps = ps_m.tile([128, 2], F32, tag="pm")
        nc.tensor.matmul(sW_ps[:], ones_row[:], srow[:], start=True, stop=True)
        sW = tiny.tile([128, 2], F32)
        nc.vector.tensor_copy(sW[:], sW_ps[:])
        rW = tiny.tile([128, 2], F32)
        nc.vector.reciprocal(rW[:], sW[:])

        # ------------- sharded ternarize + one fp8 AllGather -------------
        # Combined shard layout [1024, 2048] fp8: rows 0..511 hold the t1
        # shard ([256, 4096] -> row r maps to rows 2r/2r+1), rows 512..1023
        # the t2 shard ([512, 2048]). AllGather -> wg [8192, 2048].
        ws = dram.tile([1024, 2048], FP8, tag="ws")
        wg = dram.tile([NCORES * 1024, 2048], FP8, tag="wg", addr_space="Shared")

        def ternarize(src, nrow, width, col, tagp, dst_of):
            for sidx, (sl, i, hc) in enumerate(wchunks(src, nrow, width)):
                ch = wq.tile([128, 2048], F32, tag="wch", name=f"wb_{tagp}_{i}_{hc}")
                nc.gpsimd.dma_start(ch[:], sl)
                ob = tsb.tile([128, 2048], FP8, tag="tsb")
                if sidx % 2 == 0:
                    for q in range(4):
                        tmp = ps_m.tile([128, 512], F32, tag="pm")
                        nc.scalar.activation(tmp[:], ch[:, q * 512:(q + 1) * 512],
                                             AF.Identity, bias=biasC[:],
                                             scale=rW[:, col:col + 1])
                        nc.scalar.activation(ob[:, q * 512:(q + 1) * 512], tmp[:],
                                             AF.Sign, bias=biasNC[:], scale=1.0)
                else:
                    rwb = rW[:, col:col + 1].broadcast_to([128, 2048])
                    nc.vector.tensor_tensor(ch[:], ch[:], rwb, MULT)
                    nc.vector.tensor_scalar(ch[:], ch[:], C_MAGIC, C_MAGIC,
                                            ADD, SUB)
                    nc.vector.tensor_scalar(ob[:], ch[:], 1.0, -1.0, MIN, MAX)
                nc.sync.dma_start(dst_of(i, hc), ob[:])

        def t1_dst(i, hc):
            return (ws[i * 256:(i + 1) * 256, :]
                    .rearrange("(p two) w -> p two w", two=2)[:, hc, :])

        def t2_dst(i, hc):
            return ws[512 + i * 128:512 + (i + 1) * 128, :]

        ternarize(wus_in, 2, I, 0, "u", t1_dst)
        ternarize(wds_in, 4, H, 1, "d", t2_dst)
        nc.gpsimd.collective_compute(
            "AllGather", BYPASS, replica_groups=RG,
            ins=[ws.opt()], outs=[wg.opt()])

        t1 = t1p.tile([128, NC1 * I], FP8)  # resident ternary w_up^T
        for kc in range(NC1):
            r0 = (kc // 2) * 1024 + (kc % 2) * 256
            nc.sync.dma_start(
                t1[:, kc * I:(kc + 1) * I]
                .rearrange("p (two w) -> p two w", two=2),
                wg[r0:r0 + 256, :].rearrange("(p two) w -> p two w", two=2))

        # ------------- per-token scale helpers -------------
        def col_scales_bcast(vec):
            """[128,1] token-on-partition vector -> [128, TH] SBUF broadcast."""
            rps = ps_m.tile([1, 128], F32, tag="pm")
            nc.tensor.transpose(rps[:], vec[:], ident[:])
            row = tiny.tile([1, 128], F32, tag="row")
            nc.vector.tensor_copy(row[:], rps[:])
            bps = ps_m.tile([128, 128], F32, tag="pm")
            nc.tensor.matmul(bps[:], ones_row[:], row[:], start=True, stop=True)
            sb = scl.tile([128, TH], F32, tag="bc")
            nc.scalar.copy(sb[:], bps[:])
            return sb

        def x_half(b, j, q1):
            """Transpose + FWHT + int8 quant for token-half j of block b.

            Returns the per-token M1 vector (kept for layer-2 scales)."""
            tok0 = b * TB + j * TH
            u = fw1.tile([128, NC1 * TH], F32, tag="fw1")
            for g in range(4):
                xs = xinp.tile([128, 512], F32, tag="xin")
                nc.sync.dma_start(
                    xs[:], x_in[tok0:tok0 + TH, g * 512:(g + 1) * 512])
                pt = ps_m.tile([128, 512], F32, tag="pm")
                for k in range(4):
                    nc.tensor.transpose(
                        pt[:, k * 128:(k + 1) * 128],
                        xs[:, k * 128:(k + 1) * 128], ident[:])
                nc.scalar.copy(u[:, g * 512:(g + 1) * 512], pt[:])
            for g in range(4):
                u1 = ps_m.tile([128, 512], F32, tag="pm")
                for k in range(4):
                    c = 4 * g + k
                    nc.tensor.matmul(u1[:, k * TH:(k + 1) * TH], h128[:],
                                     u[:, c * TH:(c + 1) * TH],
                                     start=True, stop=True)
                nc.scalar.copy(u[:, g * 512:(g + 1) * 512], u1[:])
            for sg in (1, 2, 4, 8):
                _bfly_ip(nc.vector, u, NC1, sg, TH, 0.0, FR1)
                _bfly_ip(nc.gpsimd, u, NC1, sg, TH, FR1, 1.0, is_gp=True)
            # per-token absmax over [partitions x chunks]
            P1 = scl.tile([128, TH], F32, tag="p1")
            nc.vector.tensor_reduce(
                P1[:], u[:].rearrange("p (c t) -> p t c", c=NC1),
                AX, MAX, apply_absolute_value=True)
            tps = ps_m.tile([128, 128], F32, tag="pm")
            nc.tensor.transpose(tps[:], P1[:], ident[:])
            M1 = tiny.tile([128, 1], F32, tag="m1")
            nc.vector.tensor_reduce(M1[:], tps[:], AX, MAX,
                                    apply_absolute_value=True)
            nc.vector.tensor_scalar(M1[:], M1[:], ISQ1, 1e-5, MULT, MAX)
            s1t = tiny.tile([128, 1], F32, tag="s1t")
            nc.vector.reciprocal(s1t[:], M1[:])
            nc.vector.tensor_scalar(s1t[:], s1t[:], 127.0 * ISQ1, None, MULT)
            s1b = col_scales_bcast(s1t)
            uv = u[:].rearrange("p (c t) -> p c t", c=NC1)
            sbb = s1b[:, None, :].broadcast_to([128, NC1, TH])
            MS = int(NC1 * MSF)
            nc.vector.tensor_tensor(uv[:, 0:MS, :], uv[:, 0:MS, :],
                                    sbb[:, 0:MS, :], MULT)
            nc.gpsimd.tensor_tensor(uv[:, MS:NC1, :], uv[:, MS:NC1, :],
                                    sbb[:, MS:NC1, :], MULT)
            q1v = q1[:].rearrange("p (c t) -> p c t", c=NC1)
            nc.vector.tensor_scalar(
                q1v[:, :, j * TH:(j + 1) * TH], uv, C_MAGIC, C_MAGIC, ADD, SUB)
            return M1

        def gemm1(q1, rjs):
            """GEMM1 + fused relu^2 evac into per-half r tiles rjs[j]."""
            for op_ in range(NC2 // 2):
                acc = ps_g1.tile([128, 512], F32, tag="a1")
                for half in range(2):
                    oc = 2 * op_ + half
                    for cp in range(NC1):
                        nc.tensor.matmul(
                            acc[:, half * TB:(half + 1) * TB],
                            t1[:, cp * I + oc * 128: cp * I + (oc + 1) * 128],
                            q1[:, cp * TB:(cp + 1) * TB],
                            start=(cp == 0), stop=(cp == NC1 - 1))
                av = acc[:].rearrange("p (o t) -> p o t", o=2)
                for j in range(2):
                    rv = rjs[j][:].rearrange("p (m t) -> p m t", m=NC2)
                    sl = av[:, :, j * TH:(j + 1) * TH]
                    dst = rv[:, 2 * op_:2 * op_ + 2, :]
                    nc.vector.tensor_scalar(dst, sl, 0.0, None, MAX)
                    nc.scalar.activation(dst, dst, AF.Square, bias=0.0)

        def h128_2(rj):
            for g in range(NC2 // 4):
                ps = ps_m.tile([128, 512], F32, tag="pm")
                for k in range(4):
                    m = 4 * g + k
                    nc.tensor.matmul(ps[:, k * TH:(k + 1) * TH], h128[:],
                                     rj[:, m * TH:(m + 1) * TH],
                                     start=True, stop=True)
                nc.scalar.copy(rj[:, g * 512:(g + 1) * 512], ps[:])

        def quant2(rj, M1, j, q2):
            for sg in (1, 2, 4, 8, 16):
                _bfly_ip(nc.vector, rj, NC2, sg, TH, 0.0, FR2)
                _bfly_ip(nc.gpsimd, rj, NC2, sg, TH, FR2, 1.0, is_gp=True)
            # cc = (M1*sW0)^2 / (127^2 * 64)
            cc = tiny.tile([128, 1], F32, tag="cc")
            nc.vector.tensor_tensor(cc[:], M1[:], sW[:, 0:1], MULT)
            nc.vector.tensor_tensor(cc[:], cc[:], cc[:], MULT)
            nc.vector.tensor_scalar(cc[:], cc[:], 1.0 / (127.0 * 127.0 * 64.0),
                                    None, MULT)
            P2 = scl.tile([128, TH], F32, tag="p2")
            nc.vector.tensor_reduce(
                P2[:], rj[:].rearrange("p (m t) -> p t m", m=NC2),
                AX, MAX, apply_absolute_value=True)
            tps = ps_m.tile([128, 128], F32, tag="pm")
            nc.tensor.transpose(tps[:], P2[:], ident[:])
            M2 = tiny.tile([128, 1], F32, tag="m2")
            nc.vector.tensor_reduce(M2[:], tps[:], AX, MAX,
                                    apply_absolute_value=True)
            nc.vector.tensor_tensor(M2[:], M2[:], cc[:], MULT)
            nc.vector.tensor_scalar(M2[:], M2[:], 1e-5, None, MAX)
            s2t = tiny.tile([128, 1], F32, tag="s2t")
            nc.vector.reciprocal(s2t[:], M2[:])
            nc.vector.tensor_tensor(s2t[:], s2t[:], cc[:], MULT)
            nc.vector.tensor_scalar(s2t[:], s2t[:], 127.0, None, MULT)
            fb = tiny.tile([128, 1], F32, tag=f"fb{j}")
            nc.vector.tensor_tensor(fb[:], M2[:], sW[:, 1:2], MULT)
            nc.vector.tensor_scalar(fb[:], fb[:], 1.0 / 127.0, None, MULT)
            s2b = col_scales_bcast(s2t)
            rv = rj[:].rearrange("p (m t) -> p m t", m=NC2)
            sbb = s2b[:, None, :].broadcast_to([128, NC2, TH])
            MS = int(NC2 * MSF)
            nc.vector.tensor_tensor(rv[:, 0:MS, :], rv[:, 0:MS, :],
                                    sbb[:, 0:MS, :], MULT)
            nc.gpsimd.tensor_tensor(rv[:, MS:NC2, :], rv[:, MS:NC2, :],
                                    sbb[:, MS:NC2, :], MULT)
            nc.vector.tensor_scalar(q2[:], rj[:], C_MAGIC, C_MAGIC, ADD, SUB)
            return fb

        def gemm2(b, j, q2, fb):
            acc2s = [ps_g2.tile([128, 512], F32, tag="a2", name=f"a2_{hs}")
                     for hs in range(4)]
            for e in range(NC2 // 2):
                st = slab.tile([128, 2 * H], FP8, tag="t2s")
                rb = (e // 2) * 1024 + 512 + (e % 2) * 256
                nc.sync.dma_start(
                    st[:].rearrange("p (m h) -> p m h", m=2),
                    wg[rb:rb + 256, :].rearrange("(m p) h -> p m h", p=128))
                for hs in range(4):
                    for mi in range(2):
                        m2 = e * 2 + mi
                        nc.tensor.matmul(
                            acc2s[hs][:],
                            q2[:, m2 * TH:(m2 + 1) * TH],
                            st[:, mi * H + hs * 512: mi * H + (hs + 1) * 512],
                            start=(m2 == 0), stop=(m2 == NC2 - 1))
            for hs in range(4):
                ot = otp.tile([128, 512], F32, tag="ot")
                nc.scalar.activation(ot[:], acc2s[hs][:], AF.Identity,
                                     bias=0.0, scale=fb[:])
                nc.sync.dma_start(
                    out_d[b * TB + j * TH: b * TB + (j + 1) * TH,
                          hs * 512:(hs + 1) * 512], ot[:])

        # ------------- 2-deep software-pipelined block loop -------------
        # iteration k PE stream: gemm2(k-2) | xA(k+1) | gemm1(k) | h128_2(k)
        # iteration k DVE stream: bf2+quant2(k-1) | bf1+quant1(k+1) | evac(k)
        q1s = {}
        rs = {}
        m1s = {}
        q2s = {}
        fbs = {}
        q1s[0] = q1p.tile([128, NC1 * TB], BF16, tag="q1", name="q1_0")
        m1s[0] = [x_half(0, j, q1s[0]) for j in range(2)]
        for k in range(NB + 2):
            if 0 <= k - 2 < NB:
                for j in range(2):
                    gemm2(k - 2, j, q2s[(k - 2, j)], fbs[(k - 2, j)])
                    del q2s[(k - 2, j)], fbs[(k - 2, j)]
            if 0 <= k - 1 < NB:
                for j in range(2):
                    q2 = q2p.tile([128, NC2 * TH], BF16, tag="q2",
                                  name=f"q2_{k - 1}_{j}")
                    fbs[(k - 1, j)] = quant2(rs[k - 1][j], m1s[k - 1][j], j, q2)
                    q2s[(k - 1, j)] = q2
                del rs[k - 1], m1s[k - 1]
            if k + 1 < NB:
                q1s[k + 1] = q1p.tile([128, NC1 * TB], BF16, tag="q1",
                                      name=f"q1_{k + 1}")
                m1s[k + 1] = [x_half(k + 1, j, q1s[k + 1]) for j in range(2)]
            if k < NB:
                rjs = [rp.tile([128, NC2 * TH], F32, tag="rj",
                               name=f"r_{k}_{j}") for j in range(2)]
                gemm1(q1s[k], rjs)
                del q1s[k]
                h128_2(rjs[0])
                h128_2(rjs[1])
                rs[k] = rjs

    nc.finalize()
    return nc


_NC_CACHE = None


def _get_nc():
    global _NC_CACHE
    if _NC_CACHE is None:
        _NC_CACHE = build()
    return _NC_CACHE


def _hadamard128():
    h = np.array([[1.0]], dtype=np.float32)
    while h.shape[0] < 128:
        h = np.block([[h, h], [h, -h]])
    return h.astype(np.float32)


def kernel(hidden_states, w_up, w_down):
    x = np.ascontiguousarray(hidden_states.reshape(TOKENS, H), dtype=np.float32)
    wuT = np.ascontiguousarray(w_up.T, dtype=np.float32)
    wdT = np.ascontiguousarray(w_down.T, dtype=np.float32)
    h128 = _hadamard128()

    nc = _get_nc()
    in_maps = []
    for c in range(NCORES):
        in_maps.append({
            "x": x[c * T:(c + 1) * T],
            "wu_s": np.ascontiguousarray(
                wuT[c * (H // NCORES):(c + 1) * (H // NCORES)]),
            "wd_s": np.ascontiguousarray(
                wdT[c * (I // NCORES):(c + 1) * (I // NCORES)]),
            "h128": h128,
        })
    res = run_bass_kernel_spmd(nc, in_maps, list(range(NCORES))).results
    out = np.concatenate(
        [np.asarray(res[c]["out"], dtype=np.float32) for c in range(NCORES)], axis=0
    )
    return out.reshape(B, S, H)
